# revision 22
# baseline (speedup 1.0000x reference)
"""Trainium2 Bass kernel for a 2-layer GCN encoder + global mean pool.

Problem: x[100000,128] f32, edge_index[2,1600000] i64, batch[100000] i64
(sorted), W1/b1/W2/b2. Two GCNConv layers (symmetric deg^-1/2 norm, self
loops, relu) then mean-pool over 512 graphs -> [512, 128] f32.

Strategy (8 NeuronCores, data-parallel over graphs):
- Nodes partitioned by graph id into 8 contiguous shards (batch is sorted);
  each core owns the edges whose *destination* lands in its shard.
- Algebraic rewrite: aggregate-then-transform.  For each layer,
      h' = relu( (A_hat @ h) @ W + b ),   A_hat = D^-1/2 (A+I) D^-1/2
  so the sparse aggregation runs on raw 128-dim features and the dense
  matmul with W happens per 128-node window afterwards.  Self loops are
  plain edges with weight 1/deg.
- Sparse aggregation per core: per 128-edge block, dma_gather fetches the
  128 source rows (bf16, 256B each) from the feature table; a one-hot
  selection matrix M (built on the Vector engine from precomputed group-slot
  and edge-weight columns, f32 iota so slots up to 511 compare exactly)
  scatters them on the Tensor engine into a 512-wide PSUM tile covering a
  whole 4-window destination group:
      PSUM[f, 0:512] += G_blk.T @ M_blk          (M is [128, 512])
  Padding edges therefore amortize over (group, quarter) segments instead of
  (group, quarter, window), cutting padded blocks/descriptors ~13%.
- dma_gather uses int16 indices, so the node table is addressed through
  <=32768-row "quarter" slices; edges are sorted by (window-group, quarter)
  so each gather call stays quarter-pure.
- Each 128-column slice of the finished group PSUM (= Z^T per window) feeds
  the dense W matmul directly (its transposed layout is exactly the lhsT the
  Tensor engine wants); bias is added via a K=1 matmul with a ones row; relu
  on the Scalar engine.
- Between layers one 8-core AllGather shares the per-shard h1 table (bf16).
- Mean pool: the same one-hot matmul trick keyed on local graph id into a
  single resident PSUM bank, then a reciprocal-count scale.

Host-side preprocessing is purely structural (index sorting, degree counts,
normalization coefficients derived from the graph topology); all
feature/weight compute runs on device.

Tuning (measured by repeat-amplification slopes, since the axon execution
envelope hides absolute device time): the SWDGE descriptor ring is enlarged
(dynamic_dma_scratch_size 49152 = 3072 descriptors) and the gather tile pool
deepened to 16 bufs so descriptor generation overlaps transfer drain across
the 4 SWDGE queues.  Gather calls stay at 8 blocks (1024 idx) -- larger
single calls crash the SWDGE ucode.
"""

import math
import os

import numpy as np
import ml_dtypes

import concourse.bass as bass
import concourse.bacc as bacc
import concourse.mybir as mybir
import concourse.tile as tile
from concourse.bass_utils import run_bass_kernel_spmd

P = 128
C = 8               # cores
G = 512             # graphs
GPC = G // C        # graphs per core
F = 128             # feature dim (in = hid = out)
WG = 4              # windows per PSUM-resident group
SW = WG * 128       # group PSUM tile width (4 windows side by side)
QROWS = 32768       # rows per int16-addressable table slice
CALLBLK = int(os.environ.get("KERNEL_CALLBLK", "8"))
                    # max edge blocks per dma_gather call (1024 idx;
                    # >8 reportedly crashed the SWDGE ucode -- retesting)

bf16 = mybir.dt.bfloat16
f32 = mybir.dt.float32
i16 = mybir.dt.int16

BF = ml_dtypes.bfloat16


def _preprocess(x, edge_index, batch):
    """Structural preprocessing: shard nodes by graph, sort/pad edges by
    (window-group, src-quarter, dst-window), compute GCN norm weights."""
    N = x.shape[0]
    src = np.asarray(edge_index[0], dtype=np.int64)
    dst = np.asarray(edge_index[1], dtype=np.int64)
    batch = np.asarray(batch, dtype=np.int64)

    node_start = np.searchsorted(batch, np.arange(C + 1) * GPC).astype(np.int64)
    nk = np.diff(node_start)
    NODE_PAD = int(math.ceil(nk.max() / P) * P)
    NW = NODE_PAD // P
    TOT = C * NODE_PAD
    assert TOT <= 4 * QROWS
    NG = (NW + WG - 1) // WG

    core_of = (batch // GPC).astype(np.int64)
    row = (np.arange(N) - node_start[core_of] + core_of * NODE_PAD).astype(np.int64)

    deg = np.bincount(dst, minlength=N).astype(np.float64) + 1.0
    dis = 1.0 / np.sqrt(deg)

    # full edge list including self loops
    esrc = np.concatenate([src, np.arange(N)])
    edst = np.concatenate([dst, np.arange(N)])
    ew = np.concatenate([dis[src] * dis[dst], 1.0 / deg]).astype(np.float32)

    ecore = core_of[edst]
    eld = edst - node_start[ecore]
    ewin = eld >> 7
    eslot = (eld & 127).astype(np.float32)
    esrcrow = row[esrc]
    eq = (esrcrow // QROWS).astype(np.int64)      # source quarter
    eloc = (esrcrow % QROWS).astype(np.int64)     # quarter-local row
    egrp = ewin // WG

    # group-local slot: 0..WG*P-1 inside the 512-wide PSUM group tile
    gslot = (eld - egrp * WG * P).astype(np.float32)

    NQ = 4
    # segment id in (core, group, quarter) order -- blocks mix the group's
    # windows; padding is per (group, quarter) instead of per window
    seg = (ecore * NG + egrp) * NQ + eq
    NSEG = C * NG * NQ
    counts = np.bincount(seg, minlength=NSEG)
    cnt3 = counts.reshape(C, NG, NQ)
    # SPMD-common block counts per (group, quarter)
    BWS = np.ceil(cnt3.max(axis=0) / P).astype(np.int64)      # [NG, NQ]
    # every group needs >= 1 block so its PSUM tile is always written
    for g in range(NG):
        if BWS[g, :].sum() == 0:
            BWS[g, 0] = 1
    NBLK = int(BWS.sum())

    # block/segment offsets in (g, q) order
    seg_order = []          # (g, q, block_start, nblocks)
    seg_start = np.zeros((NG, NQ), np.int64)
    acc = 0
    for g in range(NG):
        for q in range(NQ):
            seg_start[g, q] = acc
            nb = int(BWS[g, q])
            if nb:
                seg_order.append((g, q, acc, nb))
            acc += nb
    assert acc == NBLK

    # scatter edges into the padded per-core layout, window-sorted within
    # each (g,q) segment so most blocks touch few windows
    order = np.argsort(seg * (NW + 1) + ewin, kind="stable")
    seg_sorted = seg[order]
    grp_excl = np.concatenate([[0], np.cumsum(counts)[:-1]])
    pos = np.arange(order.size) - grp_excl[seg_sorted]
    es = order
    dest = (ecore[es] * (NBLK * P)
            + seg_start[egrp[es], eq[es]] * P + pos)

    idx_arr = np.zeros(C * NBLK * P, np.int16)      # quarter-local src row
    slot_arr = np.zeros(C * NBLK * P, np.float32)
    w_arr = np.zeros(C * NBLK * P, np.float32)
    idx_arr[dest] = eloc[es].astype(np.int16)
    slot_arr[dest] = gslot[es]
    w_arr[dest] = ew[es]

    # per-block group-local window range of the real edges, unioned across
    # cores (the matmul column slice is program-static).  Padding edges are
    # zero rows of M, so they are harmless in any range.
    blkid = (dest % (NBLK * P)) // P
    ewi = (gslot[es] // P).astype(np.int64)
    blk_wlo = np.full(NBLK, WG - 1, np.int64)
    blk_whi = np.zeros(NBLK, np.int64)
    np.minimum.at(blk_wlo, blkid, ewi)
    np.maximum.at(blk_whi, blkid, ewi)

    # per-core uploads
    # idx: wrapped [16, NBLK*8] (logical i at [i%16, i//16]), replicated to
    # 128 partitions (the gather ucode's per-Q7-core channel groups all read
    # the same wrap)
    idx_pc = np.ascontiguousarray(
        idx_arr.reshape(C, NBLK * P // 16, 16).transpose(0, 2, 1))
    idx_pc = np.ascontiguousarray(np.tile(idx_pc, (1, 8, 1)))
    slot_pc = np.ascontiguousarray(slot_arr.reshape(C, NBLK, P).transpose(0, 2, 1))
    w_pc = np.ascontiguousarray(w_arr.reshape(C, NBLK, P).transpose(0, 2, 1))

    # node feature table, padded/bf16
    xt = np.zeros((TOT, F), BF)
    xt[row] = np.asarray(x, np.float32).astype(BF)

    # static schedule: per block -> (group, first/last-of-group);
    # gather calls: chunks of <= CALLBLK blocks within one (g, q) run.
    blk_grp = np.zeros(NBLK, np.int64)
    first_blk = {}
    last_blk = {}
    for (g, q, b0, nb) in seg_order:
        blk_grp[b0:b0 + nb] = g
        if g not in first_blk:
            first_blk[g] = b0
        last_blk[g] = b0 + nb - 1
    blk_first = np.zeros(NBLK, bool)
    blk_last = np.zeros(NBLK, bool)
    for g, b in first_blk.items():
        blk_first[b] = True
    for g, b in last_blk.items():
        blk_last[b] = True

    calls = []   # (b0, nb, quarter)
    for (g, q, b0, nb) in seg_order:
        b = b0
        while b < b0 + nb:
            k = min(CALLBLK, b0 + nb - b)
            calls.append((b, k, q))
            b += k

    # pooling metadata
    batloc = np.full((C, NODE_PAD), -1.0, np.float32)
    for c in range(C):
        nn = int(nk[c])
        batloc[c, :nn] = (batch[node_start[c]:node_start[c + 1]] - c * GPC).astype(
            np.float32)
    batloc_pc = np.ascontiguousarray(batloc.reshape(C, NW, P).transpose(0, 2, 1))

    gcnt = np.bincount(batch, minlength=G).astype(np.float32)
    counts_pc = np.ones((C, P, 1), np.float32)
    counts_pc[:, :GPC, 0] = gcnt.reshape(C, GPC)

    return dict(
        NODE_PAD=NODE_PAD, NW=NW, TOT=TOT, NBLK=NBLK,
        blk_grp=blk_grp, blk_first=blk_first, blk_last=blk_last, calls=calls,
        blk_wlo=blk_wlo, blk_whi=blk_whi,
        idx_pc=idx_pc, slot_pc=slot_pc, w_pc=w_pc, xt=xt,
        batloc_pc=batloc_pc, counts_pc=counts_pc,
    )


def _build_nc(pre):
    NW = pre["NW"]
    NBLK = pre["NBLK"]
    TOT = pre["TOT"]
    NODE_PAD = pre["NODE_PAD"]
    blk_grp = pre["blk_grp"]
    blk_wlo = pre["blk_wlo"]
    blk_whi = pre["blk_whi"]
    blk_first = pre["blk_first"]
    blk_last = pre["blk_last"]
    calls = pre["calls"]

    _nq = int(os.environ.get("KERNEL_NQUEUES", "4"))
    _scratch = int(os.environ.get("KERNEL_DMASCRATCH", "49152"))
    nc = bacc.Bacc(None, num_devices=C, num_swdge_queues=_nq,
                   dynamic_dma_scratch_size=_scratch)

    xt_d = nc.dram_tensor("xt", [TOT, F], bf16, kind="ExternalInput")
    idx_d = nc.dram_tensor("eidx", [128, NBLK * 8], i16, kind="ExternalInput")
    slot_d = nc.dram_tensor("eslot", [P, NBLK], f32, kind="ExternalInput")
    ew_d = nc.dram_tensor("ew", [P, NBLK], f32, kind="ExternalInput")
    iota_d = nc.dram_tensor("iota", [P, SW], f32, kind="ExternalInput")
    ones_d = nc.dram_tensor("ones", [1, P], bf16, kind="ExternalInput")
    w1_d = nc.dram_tensor("w1", [F, F], bf16, kind="ExternalInput")
    w2_d = nc.dram_tensor("w2", [F, F], bf16, kind="ExternalInput")
    b1_d = nc.dram_tensor("b1", [1, F], bf16, kind="ExternalInput")
    b2_d = nc.dram_tensor("b2", [1, F], bf16, kind="ExternalInput")
    batloc_d = nc.dram_tensor("batloc", [P, NW], f32, kind="ExternalInput")
    _no_gather = os.environ.get("KERNEL_NO_GATHER", "0") == "1"
    if _no_gather:
        gsrc_d = nc.dram_tensor("gsrc", [P, CALLBLK, F], bf16,
                                kind="ExternalInput")
    cnts_d = nc.dram_tensor("cnts", [P, 1], f32, kind="ExternalInput")
    out_d = nc.dram_tensor("out", [GPC, F], f32, kind="ExternalOutput")

    with tile.TileContext(nc) as tc:
        with (
            tc.tile_pool(name="const", bufs=1) as cpool,
            tc.tile_pool(name="gbuf", bufs=int(os.environ.get("KERNEL_GBUFS", "16"))) as gpool,
            tc.tile_pool(name="mt", bufs=6) as mtpool,
            tc.tile_pool(name="zt", bufs=2) as ztpool,
            tc.tile_pool(name="hsb", bufs=2) as hpool,
            tc.tile_pool(name="osb", bufs=2) as opool,
            tc.tile_pool(name="psw", bufs=4, space="PSUM") as pswpool,
            tc.tile_pool(name="psh", bufs=2, space="PSUM") as pshpool,
            tc.tile_pool(name="psp", bufs=1, space="PSUM") as psppool,
            tc.tile_pool(name="dram", bufs=1, space="DRAM") as dpool,
        ):
            # --- constants ---
            idx_sb = cpool.tile([128, NBLK * 8], i16)
            nc.sync.dma_start(out=idx_sb[:], in_=idx_d[:])
            slot_sb = cpool.tile([P, NBLK], f32)
            nc.sync.dma_start(out=slot_sb[:], in_=slot_d[:])
            ew_sb = cpool.tile([P, NBLK], f32)
            nc.sync.dma_start(out=ew_sb[:], in_=ew_d[:])
            iota_sb = cpool.tile([P, SW], f32)
            nc.sync.dma_start(out=iota_sb[:], in_=iota_d[:])
            ones_sb = cpool.tile([1, P], bf16)
            nc.sync.dma_start(out=ones_sb[:], in_=ones_d[:])
            w1_sb = cpool.tile([F, F], bf16)
            nc.sync.dma_start(out=w1_sb[:], in_=w1_d[:])
            w2_sb = cpool.tile([F, F], bf16)
            nc.sync.dma_start(out=w2_sb[:], in_=w2_d[:])
            b1_sb = cpool.tile([1, F], bf16)
            nc.sync.dma_start(out=b1_sb[:], in_=b1_d[:])
            b2_sb = cpool.tile([1, F], bf16)
            nc.sync.dma_start(out=b2_sb[:], in_=b2_d[:])
            batloc_sb = cpool.tile([P, NW], f32)
            nc.sync.dma_start(out=batloc_sb[:], in_=batloc_d[:])
            cnts_sb = cpool.tile([P, 1], f32)
            nc.sync.dma_start(out=cnts_sb[:], in_=cnts_d[:])

            # Funnel const-tile deps through the Vector engine (the ISA has a
            # small per-instruction sync-wait budget; same-engine ordering is
            # free).
            scratch = cpool.tile([P, 1], f32)
            for t in (slot_sb, ew_sb, iota_sb, w1_sb, w2_sb, batloc_sb, cnts_sb):
                nc.vector.reduce_sum(out=scratch[:], in_=t[:],
                                     axis=mybir.AxisListType.X)
            for t in (ones_sb, b1_sb, b2_sb):
                nc.vector.reduce_sum(out=scratch[:1, :], in_=t[:],
                                     axis=mybir.AxisListType.X)

            _local_tab = os.environ.get("KERNEL_LOCAL_TABLE", "0") == "1"
            _skip_l2 = os.environ.get("KERNEL_SKIP_L2", "0") == "1"
            _gather_only = os.environ.get("KERNEL_GATHER_ONLY", "0") == "1"

            pool_ps = psppool.tile([P, F], f32)

            _repeat = int(os.environ.get("KERNEL_REPEAT", "1"))
            for _rep in range(_repeat):
              h1_shard = dpool.tile([NODE_PAD, F], bf16, name=f"h1s{_rep}")
              h1_table = dpool.tile([TOT, F], bf16, addr_space="Shared",
                                    name=f"h1t{_rep}")
              if _local_tab:
                  h1_local = dpool.tile([TOT, F], bf16, name=f"h1l{_rep}")
              for layer in range(1 if _skip_l2 else 2):
                table = xt_d if layer == 0 else (
                    h1_local if _local_tab else h1_table)
                wmat_sb = w1_sb if layer == 0 else w2_sb
                b_sb = b1_sb if layer == 0 else b2_sb

                ps_tiles = {}
                for ci, (b0, nbk, q) in enumerate(calls):
                    g_t = gpool.tile([P, CALLBLK, P], bf16, tag="g")
                    if _no_gather:
                        nc.sync.dma_start(out=g_t[:, :nbk, :],
                                          in_=gsrc_d[:, :nbk, :])
                    else:
                        nc.gpsimd.dma_gather(
                            out_ap=g_t[:, :nbk, :],
                            in_ap=table[q * QROWS:min((q + 1) * QROWS, TOT), :],
                            idxs_ap=idx_sb[:, b0 * 8:(b0 + nbk) * 8],
                            num_idxs=nbk * P,
                            num_idxs_reg=nbk * P,
                            elem_size=F,
                            queue_num=ci % _nq,
                        )
                    if _gather_only:
                        nc.vector.reduce_max(
                            out=scratch[:], in_=g_t[:, 0, :],
                            axis=mybir.AxisListType.X)
                        continue
                    for j in range(nbk):
                        blk = b0 + j
                        g = int(blk_grp[blk])
                        if blk_first[blk]:
                            ps_tiles[g] = pswpool.tile(
                                [P, SW], f32, tag="psw", name=f"psw{g % 4}")
                        ps_g = ps_tiles[g]
                        # column range actually touched by this block's edges;
                        # the group's first/last blocks go full-width so the
                        # whole tile shares one start/stop accumulation cycle
                        if blk_first[blk] or blk_last[blk]:
                            lo, hi = 0, SW
                        else:
                            wl, wh = int(blk_wlo[blk]), int(blk_whi[blk])
                            if wh < wl:
                                wl = wh = 0
                            lo, hi = wl * P, (wh + 1) * P
                        mt = mtpool.tile([P, hi - lo], bf16, tag="mt")
                        nc.vector.tensor_scalar(
                            out=mt[:],
                            in0=iota_sb[:, lo:hi],
                            scalar1=slot_sb[:, blk:blk + 1],
                            scalar2=ew_sb[:, blk:blk + 1],
                            op0=mybir.AluOpType.is_equal,
                            op1=mybir.AluOpType.mult,
                        )
                        nc.tensor.matmul(
                            ps_g[:, lo:hi],
                            lhsT=g_t[:, j, :],
                            rhs=mt[:],
                            start=bool(blk_first[blk]),
                            stop=bool(blk_last[blk]),
                        )
                        if blk_last[blk]:
                            # ---- dense part for the finished 4-window group
                            zt = ztpool.tile([P, SW], bf16, tag="zt")
                            nc.vector.tensor_copy(out=zt[:], in_=ps_g[:])
                            del ps_tiles[g]
                            for wi in range(min(WG, NW - g * WG)):
                                w = g * WG + wi
                                ztw = zt[:, wi * P:(wi + 1) * P]
                                ps_h = pshpool.tile([P, F], f32, tag="psh")
                                nc.tensor.matmul(
                                    ps_h[:], lhsT=ztw, rhs=wmat_sb[:],
                                    start=True, stop=False,
                                )
                                nc.tensor.matmul(
                                    ps_h[:], lhsT=ones_sb[:], rhs=b_sb[:],
                                    start=False, stop=True,
                                )
                                h_sb = hpool.tile([P, F], bf16, tag="h")
                                nc.scalar.activation(
                                    out=h_sb[:], in_=ps_h[:],
                                    func=mybir.ActivationFunctionType.Relu,
                                )
                                if layer == 0:
                                    nc.sync.dma_start(
                                        out=h1_shard[w * P:(w + 1) * P, :],
                                        in_=h_sb[:],
                                    )
                                else:
                                    mb = mtpool.tile([P, P], bf16, tag="mb")
                                    nc.vector.tensor_scalar(
                                        out=mb[:],
                                        in0=iota_sb[:, :P],
                                        scalar1=batloc_sb[:, w:w + 1],
                                        scalar2=None,
                                        op0=mybir.AluOpType.is_equal,
                                    )
                                    nc.tensor.matmul(
                                        pool_ps[:],
                                        lhsT=mb[:],
                                        rhs=h_sb[:],
                                        start=(w == 0),
                                        stop=(w == NW - 1),
                                    )

                if layer == 0 and not _skip_l2:
                    nc.gpsimd.collective_compute(
                        "AllGather",
                        mybir.AluOpType.bypass,
                        replica_groups=[list(range(C))],
                        ins=[h1_shard[:]],
                        outs=[h1_table[:]],
                    )
                    if _local_tab:
                        nc.sync.dma_start(out=h1_local[:], in_=h1_table[:])

            # ---- finalize pool: divide by counts ----
            if _skip_l2:
                # touch pool_ps so it exists; output is meaningless
                nc.tensor.matmul(pool_ps[:], lhsT=ones_sb[:], rhs=b1_sb[:],
                                 start=True, stop=True)
            rec_sb = opool.tile([P, 1], f32, tag="rec")
            nc.vector.reciprocal(out=rec_sb[:], in_=cnts_sb[:])
            out_sb = opool.tile([P, F], f32, tag="os")
            nc.vector.tensor_scalar(
                out=out_sb[:],
                in0=pool_ps[:],
                scalar1=rec_sb[:, 0:1],
                scalar2=None,
                op0=mybir.AluOpType.mult,
            )
            nc.sync.dma_start(out=out_d[:], in_=out_sb[0:GPC, :])

    nc.compile()
    return nc


def kernel(x, edge_index, batch, W1, b1, W2, b2):
    x = np.asarray(x, np.float32)
    pre = _preprocess(x, edge_index, batch)

    iota = np.ascontiguousarray(
        np.broadcast_to(np.arange(SW, dtype=np.float32), (P, SW)))
    ones = np.ones((1, P), BF)
    w1b = np.asarray(W1, np.float32).astype(BF)
    w2b = np.asarray(W2, np.float32).astype(BF)
    b1b = np.asarray(b1, np.float32).reshape(1, F).astype(BF)
    b2b = np.asarray(b2, np.float32).reshape(1, F).astype(BF)

    in_maps = []
    for c in range(C):
        in_maps.append({
            "xt": pre["xt"],
            "eidx": pre["idx_pc"][c],
            "eslot": pre["slot_pc"][c],
            "ew": pre["w_pc"][c],
            "iota": iota,
            "ones": ones,
            "w1": w1b,
            "w2": w2b,
            "b1": b1b,
            "b2": b2b,
            "batloc": pre["batloc_pc"][c],
            "cnts": pre["counts_pc"][c],
        })

    nc = _build_nc(pre)
    res = run_bass_kernel_spmd(nc, in_maps, core_ids=list(range(C)))
    out = np.concatenate([res.results[c]["out"] for c in range(C)], axis=0)
    return out.astype(np.float32)



# revision 24
# speedup vs baseline: 1.1218x; 1.1218x over previous
"""Trainium2 Bass kernel for a 2-layer GCN encoder + global mean pool.

Problem: x[100000,128] f32, edge_index[2,1600000] i64, batch[100000] i64
(sorted), W1/b1/W2/b2. Two GCNConv layers (symmetric deg^-1/2 norm, self
loops, relu) then mean-pool over 512 graphs -> [512, 128] f32.

Strategy (8 NeuronCores, data-parallel over graphs):
- Nodes partitioned by graph id into 8 contiguous shards (batch is sorted);
  each core owns the edges whose *destination* lands in its shard.
- Algebraic rewrite: aggregate-then-transform.  For each layer,
      h' = relu( (A_hat @ h) @ W + b ),   A_hat = D^-1/2 (A+I) D^-1/2
  so the sparse aggregation runs on raw 128-dim features and the dense
  matmul with W happens per 128-node window afterwards.  Self loops are
  plain edges with weight 1/deg.
- Sparse aggregation per core: per 128-edge block, dma_gather fetches the
  128 source rows (bf16, 256B each) from the feature table; a one-hot
  selection matrix M (built on the Vector engine from precomputed group-slot
  and edge-weight columns, f32 iota so slots up to 511 compare exactly)
  scatters them on the Tensor engine into a 512-wide PSUM tile covering a
  whole 4-window destination group:
      PSUM[f, 0:512] += G_blk.T @ M_blk          (M is [128, 512])
  Padding edges therefore amortize over (group, quarter) segments instead of
  (group, quarter, window), cutting padded blocks ~10% and gather calls ~20%.
  Edges are window-sorted inside each segment and each block's matmul/one-hot
  build covers only the 128-column windows its edges touch (mean width 1.5 of
  4; the group's first/last blocks go full-width to own PSUM start/stop), so
  the wide-tile scheme costs no extra PE/Vector throughput.
- dma_gather uses int16 indices, so the node table is addressed through
  <=32768-row "quarter" slices; edges are sorted by (window-group, quarter)
  so each gather call stays quarter-pure.
- Each 128-column slice of the finished group PSUM (= Z^T per window) feeds
  the dense W matmul directly (its transposed layout is exactly the lhsT the
  Tensor engine wants); bias is added via a K=1 matmul with a ones row; relu
  on the Scalar engine.
- Between layers one 8-core AllGather shares the per-shard h1 table (bf16).
- Mean pool: the same one-hot matmul trick keyed on local graph id into a
  single resident PSUM bank, then a reciprocal-count scale.

Host-side preprocessing is purely structural (index sorting, degree counts,
normalization coefficients derived from the graph topology); all
feature/weight compute runs on device.

Tuning (measured by repeat-amplification slopes, since the axon execution
envelope hides absolute device time): the SWDGE descriptor ring is enlarged
(dynamic_dma_scratch_size 49152 = 3072 descriptors) and the gather tile pool
deepened to 16 bufs so descriptor generation overlaps transfer drain across
the 4 SWDGE queues.  Gather calls stay at 8 blocks (1024 idx) -- larger
single calls crash the SWDGE ucode.
"""

import math
import os

import numpy as np
import ml_dtypes

import concourse.bass as bass
import concourse.bacc as bacc
import concourse.mybir as mybir
import concourse.tile as tile
from concourse.bass_utils import run_bass_kernel_spmd

P = 128
C = 8               # cores
G = 512             # graphs
GPC = G // C        # graphs per core
F = 128             # feature dim (in = hid = out)
WG = 4              # windows per PSUM-resident group
SW = WG * 128       # group PSUM tile width (4 windows side by side)
QROWS = 32768       # rows per int16-addressable table slice
CALLBLK = int(os.environ.get("KERNEL_CALLBLK", "8"))
                    # max edge blocks per dma_gather call (1024 idx;
                    # >8 reportedly crashed the SWDGE ucode -- retesting)

bf16 = mybir.dt.bfloat16
f32 = mybir.dt.float32
i16 = mybir.dt.int16

BF = ml_dtypes.bfloat16


def _preprocess(x, edge_index, batch):
    """Structural preprocessing: shard nodes by graph, sort/pad edges by
    (window-group, src-quarter, dst-window), compute GCN norm weights."""
    N = x.shape[0]
    src = np.asarray(edge_index[0], dtype=np.int64)
    dst = np.asarray(edge_index[1], dtype=np.int64)
    batch = np.asarray(batch, dtype=np.int64)

    node_start = np.searchsorted(batch, np.arange(C + 1) * GPC).astype(np.int64)
    nk = np.diff(node_start)
    NODE_PAD = int(math.ceil(nk.max() / P) * P)
    NW = NODE_PAD // P
    TOT = C * NODE_PAD
    assert TOT <= 4 * QROWS
    NG = (NW + WG - 1) // WG

    core_of = (batch // GPC).astype(np.int64)
    row = (np.arange(N) - node_start[core_of] + core_of * NODE_PAD).astype(np.int64)

    deg = np.bincount(dst, minlength=N).astype(np.float64) + 1.0
    dis = 1.0 / np.sqrt(deg)

    # full edge list including self loops
    esrc = np.concatenate([src, np.arange(N)])
    edst = np.concatenate([dst, np.arange(N)])
    ew = np.concatenate([dis[src] * dis[dst], 1.0 / deg]).astype(np.float32)

    ecore = core_of[edst]
    eld = edst - node_start[ecore]
    ewin = eld >> 7
    eslot = (eld & 127).astype(np.float32)
    esrcrow = row[esrc]
    eq = (esrcrow // QROWS).astype(np.int64)      # source quarter
    eloc = (esrcrow % QROWS).astype(np.int64)     # quarter-local row
    egrp = ewin // WG

    # group-local slot: 0..WG*P-1 inside the 512-wide PSUM group tile
    gslot = (eld - egrp * WG * P).astype(np.float32)

    NQ = 4
    # segment id in (core, group, quarter) order -- blocks mix the group's
    # windows; padding is per (group, quarter) instead of per window
    seg = (ecore * NG + egrp) * NQ + eq
    NSEG = C * NG * NQ
    counts = np.bincount(seg, minlength=NSEG)
    cnt3 = counts.reshape(C, NG, NQ)
    # SPMD-common block counts per (group, quarter)
    BWS = np.ceil(cnt3.max(axis=0) / P).astype(np.int64)      # [NG, NQ]
    # every group needs >= 1 block so its PSUM tile is always written
    for g in range(NG):
        if BWS[g, :].sum() == 0:
            BWS[g, 0] = 1
    NBLK = int(BWS.sum())

    # block/segment offsets in (g, q) order
    seg_order = []          # (g, q, block_start, nblocks)
    seg_start = np.zeros((NG, NQ), np.int64)
    acc = 0
    for g in range(NG):
        for q in range(NQ):
            seg_start[g, q] = acc
            nb = int(BWS[g, q])
            if nb:
                seg_order.append((g, q, acc, nb))
            acc += nb
    assert acc == NBLK

    # scatter edges into the padded per-core layout, window-sorted within
    # each (g,q) segment so most blocks touch few windows
    order = np.argsort(seg * (NW + 1) + ewin, kind="stable")
    seg_sorted = seg[order]
    grp_excl = np.concatenate([[0], np.cumsum(counts)[:-1]])
    pos = np.arange(order.size) - grp_excl[seg_sorted]
    es = order
    dest = (ecore[es] * (NBLK * P)
            + seg_start[egrp[es], eq[es]] * P + pos)

    idx_arr = np.zeros(C * NBLK * P, np.int16)      # quarter-local src row
    slot_arr = np.zeros(C * NBLK * P, np.float32)
    w_arr = np.zeros(C * NBLK * P, np.float32)
    idx_arr[dest] = eloc[es].astype(np.int16)
    slot_arr[dest] = gslot[es]
    w_arr[dest] = ew[es]

    # per-block group-local window range of the real edges, unioned across
    # cores (the matmul column slice is program-static).  Padding edges are
    # zero rows of M, so they are harmless in any range.
    blkid = (dest % (NBLK * P)) // P
    ewi = (gslot[es] // P).astype(np.int64)
    blk_wlo = np.full(NBLK, WG - 1, np.int64)
    blk_whi = np.zeros(NBLK, np.int64)
    np.minimum.at(blk_wlo, blkid, ewi)
    np.maximum.at(blk_whi, blkid, ewi)

    # per-core uploads
    # idx: wrapped [16, NBLK*8] (logical i at [i%16, i//16]), replicated to
    # 128 partitions (the gather ucode's per-Q7-core channel groups all read
    # the same wrap)
    idx_pc = np.ascontiguousarray(
        idx_arr.reshape(C, NBLK * P // 16, 16).transpose(0, 2, 1))
    idx_pc = np.ascontiguousarray(np.tile(idx_pc, (1, 8, 1)))
    slot_pc = np.ascontiguousarray(slot_arr.reshape(C, NBLK, P).transpose(0, 2, 1))
    w_pc = np.ascontiguousarray(w_arr.reshape(C, NBLK, P).transpose(0, 2, 1))

    # node feature table, padded/bf16
    xt = np.zeros((TOT, F), BF)
    xt[row] = np.asarray(x, np.float32).astype(BF)

    # static schedule: per block -> (group, first/last-of-group);
    # gather calls: chunks of <= CALLBLK blocks within one (g, q) run.
    blk_grp = np.zeros(NBLK, np.int64)
    first_blk = {}
    last_blk = {}
    for (g, q, b0, nb) in seg_order:
        blk_grp[b0:b0 + nb] = g
        if g not in first_blk:
            first_blk[g] = b0
        last_blk[g] = b0 + nb - 1
    blk_first = np.zeros(NBLK, bool)
    blk_last = np.zeros(NBLK, bool)
    for g, b in first_blk.items():
        blk_first[b] = True
    for g, b in last_blk.items():
        blk_last[b] = True

    calls = []   # (b0, nb, quarter)
    for (g, q, b0, nb) in seg_order:
        b = b0
        while b < b0 + nb:
            k = min(CALLBLK, b0 + nb - b)
            calls.append((b, k, q))
            b += k

    # pooling metadata
    batloc = np.full((C, NODE_PAD), -1.0, np.float32)
    for c in range(C):
        nn = int(nk[c])
        batloc[c, :nn] = (batch[node_start[c]:node_start[c + 1]] - c * GPC).astype(
            np.float32)
    batloc_pc = np.ascontiguousarray(batloc.reshape(C, NW, P).transpose(0, 2, 1))

    gcnt = np.bincount(batch, minlength=G).astype(np.float32)
    counts_pc = np.ones((C, P, 1), np.float32)
    counts_pc[:, :GPC, 0] = gcnt.reshape(C, GPC)

    return dict(
        NODE_PAD=NODE_PAD, NW=NW, TOT=TOT, NBLK=NBLK,
        blk_grp=blk_grp, blk_first=blk_first, blk_last=blk_last, calls=calls,
        blk_wlo=blk_wlo, blk_whi=blk_whi,
        idx_pc=idx_pc, slot_pc=slot_pc, w_pc=w_pc, xt=xt,
        batloc_pc=batloc_pc, counts_pc=counts_pc,
    )


def _build_nc(pre):
    NW = pre["NW"]
    NBLK = pre["NBLK"]
    TOT = pre["TOT"]
    NODE_PAD = pre["NODE_PAD"]
    blk_grp = pre["blk_grp"]
    blk_wlo = pre["blk_wlo"]
    blk_whi = pre["blk_whi"]
    blk_first = pre["blk_first"]
    blk_last = pre["blk_last"]
    calls = pre["calls"]

    _nq = int(os.environ.get("KERNEL_NQUEUES", "4"))
    _scratch = int(os.environ.get("KERNEL_DMASCRATCH", "49152"))
    nc = bacc.Bacc(None, num_devices=C, num_swdge_queues=_nq,
                   dynamic_dma_scratch_size=_scratch)

    xt_d = nc.dram_tensor("xt", [TOT, F], bf16, kind="ExternalInput")
    idx_d = nc.dram_tensor("eidx", [128, NBLK * 8], i16, kind="ExternalInput")
    slot_d = nc.dram_tensor("eslot", [P, NBLK], f32, kind="ExternalInput")
    ew_d = nc.dram_tensor("ew", [P, NBLK], f32, kind="ExternalInput")
    iota_d = nc.dram_tensor("iota", [P, SW], f32, kind="ExternalInput")
    ones_d = nc.dram_tensor("ones", [1, P], bf16, kind="ExternalInput")
    w1_d = nc.dram_tensor("w1", [F, F], bf16, kind="ExternalInput")
    w2_d = nc.dram_tensor("w2", [F, F], bf16, kind="ExternalInput")
    b1_d = nc.dram_tensor("b1", [1, F], bf16, kind="ExternalInput")
    b2_d = nc.dram_tensor("b2", [1, F], bf16, kind="ExternalInput")
    batloc_d = nc.dram_tensor("batloc", [P, NW], f32, kind="ExternalInput")
    _no_gather = os.environ.get("KERNEL_NO_GATHER", "0") == "1"
    if _no_gather:
        gsrc_d = nc.dram_tensor("gsrc", [P, CALLBLK, F], bf16,
                                kind="ExternalInput")
    cnts_d = nc.dram_tensor("cnts", [P, 1], f32, kind="ExternalInput")
    out_d = nc.dram_tensor("out", [GPC, F], f32, kind="ExternalOutput")

    with tile.TileContext(nc) as tc:
        with (
            tc.tile_pool(name="const", bufs=1) as cpool,
            tc.tile_pool(name="gbuf", bufs=int(os.environ.get("KERNEL_GBUFS", "16"))) as gpool,
            tc.tile_pool(name="mt", bufs=int(os.environ.get("KERNEL_MTBUFS", "6"))) as mtpool,
            tc.tile_pool(name="zt", bufs=2) as ztpool,
            tc.tile_pool(name="hsb", bufs=2) as hpool,
            tc.tile_pool(name="osb", bufs=2) as opool,
            tc.tile_pool(name="psw", bufs=int(os.environ.get("KERNEL_PSWBUFS", "4")), space="PSUM") as pswpool,
            tc.tile_pool(name="psh", bufs=2, space="PSUM") as pshpool,
            tc.tile_pool(name="psp", bufs=1, space="PSUM") as psppool,
            tc.tile_pool(name="dram", bufs=1, space="DRAM") as dpool,
        ):
            # --- constants ---
            idx_sb = cpool.tile([128, NBLK * 8], i16)
            nc.sync.dma_start(out=idx_sb[:], in_=idx_d[:])
            slot_sb = cpool.tile([P, NBLK], f32)
            nc.sync.dma_start(out=slot_sb[:], in_=slot_d[:])
            ew_sb = cpool.tile([P, NBLK], f32)
            nc.sync.dma_start(out=ew_sb[:], in_=ew_d[:])
            iota_sb = cpool.tile([P, SW], f32)
            nc.sync.dma_start(out=iota_sb[:], in_=iota_d[:])
            ones_sb = cpool.tile([1, P], bf16)
            nc.sync.dma_start(out=ones_sb[:], in_=ones_d[:])
            w1_sb = cpool.tile([F, F], bf16)
            nc.sync.dma_start(out=w1_sb[:], in_=w1_d[:])
            w2_sb = cpool.tile([F, F], bf16)
            nc.sync.dma_start(out=w2_sb[:], in_=w2_d[:])
            b1_sb = cpool.tile([1, F], bf16)
            nc.sync.dma_start(out=b1_sb[:], in_=b1_d[:])
            b2_sb = cpool.tile([1, F], bf16)
            nc.sync.dma_start(out=b2_sb[:], in_=b2_d[:])
            batloc_sb = cpool.tile([P, NW], f32)
            nc.sync.dma_start(out=batloc_sb[:], in_=batloc_d[:])
            cnts_sb = cpool.tile([P, 1], f32)
            nc.sync.dma_start(out=cnts_sb[:], in_=cnts_d[:])

            # Funnel const-tile deps through the Vector engine (the ISA has a
            # small per-instruction sync-wait budget; same-engine ordering is
            # free).
            scratch = cpool.tile([P, 1], f32)
            for t in (slot_sb, ew_sb, iota_sb, w1_sb, w2_sb, batloc_sb, cnts_sb):
                nc.vector.reduce_sum(out=scratch[:], in_=t[:],
                                     axis=mybir.AxisListType.X)
            for t in (ones_sb, b1_sb, b2_sb):
                nc.vector.reduce_sum(out=scratch[:1, :], in_=t[:],
                                     axis=mybir.AxisListType.X)

            _local_tab = os.environ.get("KERNEL_LOCAL_TABLE", "0") == "1"
            _skip_l2 = os.environ.get("KERNEL_SKIP_L2", "0") == "1"
            _gather_only = os.environ.get("KERNEL_GATHER_ONLY", "0") == "1"

            pool_ps = psppool.tile([P, F], f32)

            _repeat = int(os.environ.get("KERNEL_REPEAT", "1"))
            for _rep in range(_repeat):
              h1_shard = dpool.tile([NODE_PAD, F], bf16, name=f"h1s{_rep}")
              h1_table = dpool.tile([TOT, F], bf16, addr_space="Shared",
                                    name=f"h1t{_rep}")
              if _local_tab:
                  h1_local = dpool.tile([TOT, F], bf16, name=f"h1l{_rep}")
              for layer in range(1 if _skip_l2 else 2):
                table = xt_d if layer == 0 else (
                    h1_local if _local_tab else h1_table)
                wmat_sb = w1_sb if layer == 0 else w2_sb
                b_sb = b1_sb if layer == 0 else b2_sb

                ps_tiles = {}
                for ci, (b0, nbk, q) in enumerate(calls):
                    g_t = gpool.tile([P, CALLBLK, P], bf16, tag="g")
                    if _no_gather:
                        nc.sync.dma_start(out=g_t[:, :nbk, :],
                                          in_=gsrc_d[:, :nbk, :])
                    else:
                        nc.gpsimd.dma_gather(
                            out_ap=g_t[:, :nbk, :],
                            in_ap=table[q * QROWS:min((q + 1) * QROWS, TOT), :],
                            idxs_ap=idx_sb[:, b0 * 8:(b0 + nbk) * 8],
                            num_idxs=nbk * P,
                            num_idxs_reg=nbk * P,
                            elem_size=F,
                            queue_num=ci % _nq,
                        )
                    if _gather_only:
                        nc.vector.reduce_max(
                            out=scratch[:], in_=g_t[:, 0, :],
                            axis=mybir.AxisListType.X)
                        continue
                    for j in range(nbk):
                        blk = b0 + j
                        g = int(blk_grp[blk])
                        if blk_first[blk]:
                            ps_tiles[g] = pswpool.tile(
                                [P, SW], f32, tag="psw", name=f"psw{g % 4}")
                        ps_g = ps_tiles[g]
                        # column range actually touched by this block's edges;
                        # the group's first/last blocks go full-width so the
                        # whole tile shares one start/stop accumulation cycle
                        if blk_first[blk] or blk_last[blk]:
                            lo, hi = 0, SW
                        else:
                            wl, wh = int(blk_wlo[blk]), int(blk_whi[blk])
                            if wh < wl:
                                wl = wh = 0
                            lo, hi = wl * P, (wh + 1) * P
                        mt = mtpool.tile([P, hi - lo], bf16, tag="mt")
                        nc.vector.tensor_scalar(
                            out=mt[:],
                            in0=iota_sb[:, lo:hi],
                            scalar1=slot_sb[:, blk:blk + 1],
                            scalar2=ew_sb[:, blk:blk + 1],
                            op0=mybir.AluOpType.is_equal,
                            op1=mybir.AluOpType.mult,
                        )
                        nc.tensor.matmul(
                            ps_g[:, lo:hi],
                            lhsT=g_t[:, j, :],
                            rhs=mt[:],
                            start=bool(blk_first[blk]),
                            stop=bool(blk_last[blk]),
                        )
                        if blk_last[blk]:
                            # ---- dense part for the finished 4-window group
                            zt = ztpool.tile([P, SW], bf16, tag="zt")
                            nc.vector.tensor_copy(out=zt[:], in_=ps_g[:])
                            del ps_tiles[g]
                            for wi in range(min(WG, NW - g * WG)):
                                w = g * WG + wi
                                ztw = zt[:, wi * P:(wi + 1) * P]
                                ps_h = pshpool.tile([P, F], f32, tag="psh")
                                nc.tensor.matmul(
                                    ps_h[:], lhsT=ztw, rhs=wmat_sb[:],
                                    start=True, stop=False,
                                )
                                nc.tensor.matmul(
                                    ps_h[:], lhsT=ones_sb[:], rhs=b_sb[:],
                                    start=False, stop=True,
                                )
                                h_sb = hpool.tile([P, F], bf16, tag="h")
                                nc.scalar.activation(
                                    out=h_sb[:], in_=ps_h[:],
                                    func=mybir.ActivationFunctionType.Relu,
                                )
                                if layer == 0:
                                    nc.sync.dma_start(
                                        out=h1_shard[w * P:(w + 1) * P, :],
                                        in_=h_sb[:],
                                    )
                                else:
                                    mb = mtpool.tile([P, P], bf16, tag="mb")
                                    nc.vector.tensor_scalar(
                                        out=mb[:],
                                        in0=iota_sb[:, :P],
                                        scalar1=batloc_sb[:, w:w + 1],
                                        scalar2=None,
                                        op0=mybir.AluOpType.is_equal,
                                    )
                                    nc.tensor.matmul(
                                        pool_ps[:],
                                        lhsT=mb[:],
                                        rhs=h_sb[:],
                                        start=(w == 0),
                                        stop=(w == NW - 1),
                                    )

                if layer == 0 and not _skip_l2:
                    nc.gpsimd.collective_compute(
                        "AllGather",
                        mybir.AluOpType.bypass,
                        replica_groups=[list(range(C))],
                        ins=[h1_shard[:]],
                        outs=[h1_table[:]],
                    )
                    if _local_tab:
                        nc.sync.dma_start(out=h1_local[:], in_=h1_table[:])

            # ---- finalize pool: divide by counts ----
            if _skip_l2:
                # touch pool_ps so it exists; output is meaningless
                nc.tensor.matmul(pool_ps[:], lhsT=ones_sb[:], rhs=b1_sb[:],
                                 start=True, stop=True)
            rec_sb = opool.tile([P, 1], f32, tag="rec")
            nc.vector.reciprocal(out=rec_sb[:], in_=cnts_sb[:])
            out_sb = opool.tile([P, F], f32, tag="os")
            nc.vector.tensor_scalar(
                out=out_sb[:],
                in0=pool_ps[:],
                scalar1=rec_sb[:, 0:1],
                scalar2=None,
                op0=mybir.AluOpType.mult,
            )
            nc.sync.dma_start(out=out_d[:], in_=out_sb[0:GPC, :])

    nc.compile()
    return nc


def kernel(x, edge_index, batch, W1, b1, W2, b2):
    x = np.asarray(x, np.float32)
    pre = _preprocess(x, edge_index, batch)

    iota = np.ascontiguousarray(
        np.broadcast_to(np.arange(SW, dtype=np.float32), (P, SW)))
    ones = np.ones((1, P), BF)
    w1b = np.asarray(W1, np.float32).astype(BF)
    w2b = np.asarray(W2, np.float32).astype(BF)
    b1b = np.asarray(b1, np.float32).reshape(1, F).astype(BF)
    b2b = np.asarray(b2, np.float32).reshape(1, F).astype(BF)

    in_maps = []
    for c in range(C):
        in_maps.append({
            "xt": pre["xt"],
            "eidx": pre["idx_pc"][c],
            "eslot": pre["slot_pc"][c],
            "ew": pre["w_pc"][c],
            "iota": iota,
            "ones": ones,
            "w1": w1b,
            "w2": w2b,
            "b1": b1b,
            "b2": b2b,
            "batloc": pre["batloc_pc"][c],
            "cnts": pre["counts_pc"][c],
        })

    nc = _build_nc(pre)
    res = run_bass_kernel_spmd(nc, in_maps, core_ids=list(range(C)))
    out = np.concatenate([res.results[c]["out"] for c in range(C)], axis=0)
    return out.astype(np.float32)

